# revision 28
# baseline (speedup 1.0000x reference)
"""Trainium2 Bass kernel for local self-attention with DeBERTa-style relative
position biases (banded attention, window W=128, S=2048, H=16 heads, d=64).

Sharding: pure sequence sharding across 8 cores. Core c owns queries
[256c, 256c+256) and computes all 16 heads for them, including the output
projection rows. No collectives; host concatenates row blocks.

Per-core layout (local coords):
  - kv window: global [256c-128, 256c+384) -> 512 keys, zero-padded at sequence
    edges (host pads hiddenT).
  - q blocks: 2 blocks of 128 queries; q block b sees local keys
    [128b, 128b+384) = local key blocks {b, b+1, b+2}.
  - relidx (relative position index, 0..256):
      c2p: score[i, j] += c2p[q=i, j - i]        (j = window col)
      p2c: score[i, j] += p2c[key, i - key + 256] (key = local key index)
  - The skews are materialized by bouncing compact tables through a DRAM
    scratch laid out as [16 heads, 6 blocks, 130, 384] fp16: blocks 0-3 are
    p2c per key block, blocks 4-5 are c2p per q block. Data rows live at
    storage rows 1..129; row 0 and columns 257..384 of the c2p blocks hold
    NEG so the strided skew re-read lands on NEG exactly at band-invalid
    positions (the mask is built into the bounce).
  - Sequence-edge masking is folded into p2c compact tiles via a
    per-partition bias add of the edge_mask input.
  - DMA instruction count is minimized (HWDGE has ~625ns fixed cost per
    dma_start): one merged write per head, merged 3-dim skew reads, merged
    const loads; some transfers are issued on the gpsimd SWDGE path to
    spread descriptor-generation cost across engines.

All matmuls fp16 inputs with fp32 PSUM accumulation (validated ~0.1% rel err).
"""

import sys
from contextlib import ExitStack

import numpy as np

sys.path.insert(0, "/opt/trn_rl_repo")

import concourse.bass as bass
import concourse.mybir as mybir
import concourse.tile as tile
from concourse import bacc

F32 = mybir.dt.float32
F16 = mybir.dt.float16

S, HD, H, D, W = 2048, 1024, 16, 64, 128
NC = 8            # cores
SQ = S // NC      # queries per core (256)
SK = SQ + 2 * W   # kv window per core (512)
NB = SQ // 128    # q blocks per core (2)
NKB = SK // 128   # key blocks per core (4)
NCH = HD // 128   # 128-chunks of hidden dim (8)
R = 2 * W + 1     # 257 relative positions
NEG = -30000.0    # fp16-representable; exp() underflows to 0
T = 384           # c2p bounce row stride
BLK = 130 * T     # elements per c2p bounce block
BLKP = 130 * R    # elements per p2c bounce block (257-stride rows)
HBLK = 4 * BLKP + 2 * BLK   # elements per head in the bounce


def build_nc(loop_n=1):
    nc = bacc.Bacc("TRN2", target_bir_lowering=False, debug=False)

    ap_in = lambda name, shape, dt: nc.dram_tensor(name, shape, dt, kind="ExternalInput").ap()
    hT = ap_in("hT", [NCH, 128, SK], F16)          # hiddenT window chunks
    wqT = ap_in("wqT", [NCH, 128, HD], F16)        # Wq.T chunks (contraction = partition)
    wkT = ap_in("wkT", [NCH, 128, HD], F16)
    wvT = ap_in("wvT", [NCH, 128, HD], F16)
    woT = ap_in("woT", [NCH, 128, HD], F16)        # Wo.T chunks (ctx-dim = partition)
    pk = ap_in("pk", [NCH, 128, R], F16)           # position_key per head-pair
    pq = ap_in("pq", [NCH, 128, R], F16)           # position_query per head-pair
    em = ap_in("em", [128, NKB], F32)              # edge mask (per local key)
    bo_in = ap_in("bo", [HD], F32)
    iden = ap_in("iden", [128, 128], F16)
    out = nc.dram_tensor("out", [SQ, HD], F32, kind="ExternalOutput").ap()

    with tile.TileContext(nc) as tc, ExitStack() as ctx:
        consts = ctx.enter_context(tc.tile_pool(name="consts", bufs=1))
        persist = ctx.enter_context(tc.tile_pool(name="persist", bufs=1))
        dram = ctx.enter_context(tc.tile_pool(name="dram", bufs=1, space="DRAM"))

        # ---- load constants (one DMA per tensor via 3-dim APs) ----
        def load_merged(src, free, tag, chunk_free):
            t = consts.tile([128, NCH * chunk_free], F16, tag=tag, name=tag)
            src_ap = bass.AP(tensor=src.tensor, offset=0,
                             ap=[[chunk_free, 128], [128 * chunk_free, NCH], [1, chunk_free]])
            nc.sync.dma_start(out=t, in_=src_ap)
            return t

        hT_sb = load_merged(hT, SK, "hTm", SK)          # [128, 8*512]
        wq_sb = load_merged(wqT, HD, "wqm", HD)         # [128, 8*1024]
        wk_sb = load_merged(wkT, HD, "wkm", HD)
        wv_sb = load_merged(wvT, HD, "wvm", HD)
        wo_sb = load_merged(woT, HD, "wom", HD)
        pk_sb = load_merged(pk, R, "pkm", R)            # [128, 8*257]
        pq_sb = load_merged(pq, R, "pqm", R)
        em_sb = consts.tile([128, NKB], F32)
        nc.sync.dma_start(out=em_sb, in_=em)
        id_sb = consts.tile([128, 128], F16)
        nc.sync.dma_start(out=id_sb, in_=iden)
        bo_sb = consts.tile([128, HD], F32)
        nc.sync.dma_start(out=bo_sb, in_=bass.AP(tensor=bo_in.tensor, offset=0, ap=[[0, 128], [1, HD]]))
        negrow = consts.tile([128, T], F16)
        nc.gpsimd.memset(negrow, NEG)

        hTc = lambda c: hT_sb[:, c * SK:(c + 1) * SK]
        wqc = lambda c: wq_sb[:, c * HD:(c + 1) * HD]
        wkc = lambda c: wk_sb[:, c * HD:(c + 1) * HD]
        wvc = lambda c: wv_sb[:, c * HD:(c + 1) * HD]
        woc = lambda c: wo_sb[:, c * HD:(c + 1) * HD]
        pkc = lambda c: pk_sb[:, c * R:(c + 1) * R]
        pqc = lambda c: pq_sb[:, c * R:(c + 1) * R]

        # ---- persistent activation tiles ----
        qT_sb = [persist.tile([128, SQ], F16, tag=f"qT{i}", name=f"qT{i}") for i in range(NCH)]
        kT_sb = [persist.tile([128, SK], F16, tag=f"kT{i}", name=f"kT{i}") for i in range(NCH)]
        v_sb = [persist.tile([128, HD], F16, tag=f"v{i}", name=f"v{i}") for i in range(NKB)]
        ctxT_sb = [persist.tile([128, SQ], F16, tag=f"ctxT{i}", name=f"ctxT{i}") for i in range(NCH)]

        # DRAM bounce scratch: [16 heads, 6 blocks, 130, 384] f16
        bnc = dram.tile([H * HBLK], F16, name="bnc")

        # ---- optional benchmark loop around the whole computation ----
        loop_cm = tc.For_i(0, loop_n, 1) if loop_n > 1 else None
        if loop_cm is not None:
            ctx.enter_context(loop_cm)

        # NEG pad row for every c2p block: two DMAs, one per c2p block index
        for b in range(NB):
            dst = bass.AP(tensor=bnc.tensor, offset=bnc.offset + 4 * BLKP + b * BLK,
                          ap=[[HBLK, H], [1, T]])
            nc.gpsimd.dma_start(out=dst, in_=negrow[0:H, :])
        # row 0 of every p2c block: skew reads touch it at band-invalid spots;
        # must be finite (NaN garbage would poison the scores)
        dst = bass.AP(tensor=bnc.tensor, offset=bnc.offset,
                      ap=[[HBLK, H], [BLKP, NKB], [1, R]])
        nc.gpsimd.dma_start(out=dst, in_=negrow[0:H * NKB, 0:R])

        # ---- phases A+B+C interleaved per head-pair chunk ----
        with tc.tile_pool(name="psA", bufs=2, space="PSUM") as psA, \
             tc.tile_pool(name="psB", bufs=2, space="PSUM") as psB, \
             tc.tile_pool(name="psS", bufs=2, space="PSUM") as psS, \
             tc.tile_pool(name="psP", bufs=1, space="PSUM") as psP, \
             tc.tile_pool(name="psX", bufs=1, space="PSUM") as psX, \
             tc.tile_pool(name="wrk", bufs=4) as wrk, \
             tc.tile_pool(name="gpool", bufs=6) as gpool, \
             tc.tile_pool(name="sm", bufs=4) as sm:

            def emit_v(n):
                for kb in range(NKB):
                    ps = psA.tile([128, 512], F32, tag="a", name="psv")
                    for c in range(NCH):
                        nc.tensor.matmul(ps, lhsT=hTc(c)[:, kb * 128:(kb + 1) * 128],
                                         rhs=wvc(c)[:, n * 512:(n + 1) * 512],
                                         start=(c == 0), stop=(c == NCH - 1))
                    nc.vector.tensor_copy(v_sb[kb][:, n * 512:(n + 1) * 512], ps)

            def emit_attention(h):
                ch, hf = h // 2, 64 * (h % 2)
                hbase = bnc.offset + h * HBLK
                stp = wrk.tile([128, NKB, R], F16, tag="stp", name="stp")
                for kb in range(NKB):
                    ps = psB.tile([128, R], F32, tag="b", name="psb")
                    nc.tensor.matmul(ps, lhsT=kT_sb[ch][hf:hf + 64, kb * 128:(kb + 1) * 128],
                                     rhs=pqc(ch)[hf:hf + 64, :], start=True, stop=True)
                    nc.scalar.activation(stp[:, kb, :], ps, mybir.ActivationFunctionType.Identity,
                                         bias=em_sb[:, kb:kb + 1])
                stc = wrk.tile([128, NB, T], F16, tag="stc", name="stc")
                for b in range(NB):
                    ps = psB.tile([128, R], F32, tag="b", name="psb")
                    nc.tensor.matmul(ps, lhsT=qT_sb[ch][hf:hf + 64, b * 128:(b + 1) * 128],
                                     rhs=pkc(ch)[hf:hf + 64, :], start=True, stop=True)
                    nc.vector.tensor_copy(stc[:, b, 0:R], ps)
                    nc.gpsimd.memset(stc[:, b, R:T], NEG)
                dstp = bass.AP(tensor=bnc.tensor, offset=hbase + R,
                               ap=[[R, 128], [BLKP, NKB], [1, R]])
                nc.sync.dma_start(out=dstp, in_=stp)
                dstc = bass.AP(tensor=bnc.tensor, offset=hbase + 4 * BLKP + T,
                               ap=[[T, 128], [BLK, NB], [1, T]])
                nc.gpsimd.dma_start(out=dstc, in_=stc)

                cg = gpool.tile([128, NB, T], F16, tag="cg", name="cg")
                srcap = bass.AP(tensor=bnc.tensor, offset=hbase + 4 * BLKP + T,
                                ap=[[T - 1, 128], [BLK, NB], [1, T]])
                nc.gpsimd.dma_start(out=cg, in_=srcap)
                gb = []
                for b in range(NB):
                    g = gpool.tile([128, 3, 128], F16, tag="gb", name="gb")
                    srcap = bass.AP(tensor=bnc.tensor,
                                    offset=hbase + b * BLKP + (R + 256),
                                    ap=[[R - 1, 128], [BLKP - 128, 3], [1, 128]])
                    nc.sync.dma_start(out=g, in_=srcap)
                    gb.append(g)

                for b in range(NB):
                    # content + c2p (identity-matmul add) + p2c (transpose as a
                    # normal matmul: out = gb_sub.T @ I) all accumulate in PSUM
                    score = psS.tile([128, 384], F32, tag="s", name="score")
                    nc.tensor.matmul(score, lhsT=qT_sb[ch][hf:hf + 64, b * 128:(b + 1) * 128],
                                     rhs=kT_sb[ch][hf:hf + 64, b * 128:b * 128 + 384],
                                     start=True, stop=True)
                    for t in range(3):
                        nc.tensor.matmul(score[:, t * 128:(t + 1) * 128],
                                         lhsT=gb[b][:, t, :], rhs=id_sb,
                                         start=False, stop=True, skip_group_check=True)
                    s2 = wrk.tile([128, 384], F32, tag="s2", name="s2")
                    nc.vector.tensor_tensor(s2, score, cg[:, b, :], op=mybir.AluOpType.add)
                    pr = wrk.tile([128, 384], F32, tag="pr", name="pr")
                    sums = sm.tile([128, 1], F32, tag="sums", name="sums")
                    nc.scalar.activation(pr, s2, mybir.ActivationFunctionType.Exp,
                                         accum_out=sums)
                    rec = sm.tile([128, 1], F32, tag="rec", name="rec")
                    nc.vector.reciprocal(rec, sums)
                    pn = wrk.tile([128, 384], F16, tag="pn", name="pn")
                    nc.vector.tensor_scalar_mul(pn, pr, rec)
                    prT = psP.tile([128, 384], F16, tag="p", name="prT")
                    for t in range(3):
                        nc.tensor.matmul(prT[:, t * 128:(t + 1) * 128],
                                         lhsT=pn[:, t * 128:(t + 1) * 128],
                                         rhs=id_sb, is_transpose=True,
                                         start=True, stop=True, skip_group_check=True)
                    pnT = wrk.tile([128, 384], F16, tag="pnT", name="pnT")
                    nc.vector.tensor_copy(pnT, prT)
                    cx = psX.tile([64, 128], F32, tag="x", name="cx")
                    for t in range(3):
                        nc.tensor.matmul(cx, lhsT=v_sb[b + t][:, h * 64:h * 64 + 64],
                                         rhs=pnT[:, t * 128:(t + 1) * 128],
                                         start=(t == 0), stop=(t == 2))
                    nc.vector.tensor_copy(ctxT_sb[ch][hf:hf + 64, b * 128:(b + 1) * 128], cx)

            for ch in range(NCH):
                ps = psA.tile([128, SQ], F32, tag="a", name="psq")
                for c in range(NCH):
                    nc.tensor.matmul(ps, lhsT=wqc(c)[:, ch * 128:(ch + 1) * 128],
                                     rhs=hTc(c)[:, W:W + SQ], start=(c == 0), stop=(c == NCH - 1))
                nc.vector.tensor_copy(qT_sb[ch], ps)
                ps = psA.tile([128, SK], F32, tag="a", name="psk")
                for c in range(NCH):
                    nc.tensor.matmul(ps, lhsT=wkc(c)[:, ch * 128:(ch + 1) * 128],
                                     rhs=hTc(c), start=(c == 0), stop=(c == NCH - 1))
                nc.vector.tensor_copy(kT_sb[ch], ps)
                if ch == 0:
                    emit_v(0)
                if ch == 4:
                    emit_v(1)
                emit_attention(2 * ch)
                emit_attention(2 * ch + 1)

        # ---- phase D: output projection ----
        with tc.tile_pool(name="psD", bufs=2, space="PSUM") as psD, \
             tc.tile_pool(name="owrk", bufs=1) as owrk:
            ob = owrk.tile([128, NB, HD], F32, name="ob")
            for sb_i in range(NB):
                for n in range(2):
                    ps = psD.tile([128, 512], F32, tag="d")
                    for c in range(NCH):
                        nc.tensor.matmul(ps, lhsT=ctxT_sb[c][:, sb_i * 128:(sb_i + 1) * 128],
                                         rhs=woc(c)[:, n * 512:(n + 1) * 512],
                                         start=(c == 0), stop=(c == NCH - 1))
                    nc.vector.tensor_tensor(ob[:, sb_i, n * 512:(n + 1) * 512], ps,
                                            bo_sb[:, n * 512:(n + 1) * 512],
                                            op=mybir.AluOpType.add)
            dst = bass.AP(tensor=out.tensor, offset=0,
                          ap=[[HD, 128], [128 * HD, NB], [1, HD]])
            nc.sync.dma_start(out=dst, in_=ob)
    nc.compile()
    return nc


def make_core_inputs(hidden_states, Wq, Wk, Wv, Wo, position_query, position_key, bo):
    """Host-side sharding/layout prep. Returns list of 8 per-core input dicts."""
    f16 = np.float16
    hT = np.ascontiguousarray(hidden_states[0].astype(np.float32).T)   # [HD, S]
    hT_pad = np.zeros((HD, S + 2 * W), np.float32)
    hT_pad[:, W:W + S] = hT

    def chunks(x):   # [HD, N] -> [NCH, 128, N]
        return np.ascontiguousarray(x.reshape(NCH, 128, -1))

    wq_h = chunks(Wq.astype(np.float32).T).astype(f16)
    wk_h = chunks(Wk.astype(np.float32).T).astype(f16)
    wv_h = chunks(Wv.astype(np.float32).T).astype(f16)
    wo_h = chunks(Wo.astype(np.float32).T).astype(f16)
    # position tables: head-pair chunks, [NCH, 128, R] with partition = 64*(h%2)+d
    pk_h = position_key[0].astype(np.float32).reshape(NCH, 128, R).astype(f16)
    pq_h = position_query[0].astype(np.float32).reshape(NCH, 128, R).astype(f16)
    iden = np.eye(128, dtype=f16)
    bo_h = bo.astype(np.float32)

    in_maps = []
    for c in range(NC):
        lo = c * SQ                      # global query start
        win = hT_pad[:, lo:lo + SK]      # padded coords: global key lo-W+j at col lo+j
        em = np.zeros((128, NKB), np.float32)
        for kb in range(NKB):
            keys = lo - W + kb * 128 + np.arange(128)
            em[(keys < 0) | (keys >= S), kb] = NEG
        in_maps.append({
            "hT": chunks(win).astype(f16),
            "wqT": wq_h, "wkT": wk_h, "wvT": wv_h, "woT": wo_h,
            "pk": pk_h, "pq": pq_h,
            "em": em, "bo": bo_h, "iden": iden,
        })
    return in_maps


_NC_CACHE = {}


def kernel(hidden_states, attention_mask, Wq, Wk, Wv, position_query, position_key,
           Wo, bo, **_unused):
    from concourse.bass_utils import run_bass_kernel_spmd

    in_maps = make_core_inputs(np.asarray(hidden_states), np.asarray(Wq), np.asarray(Wk),
                               np.asarray(Wv), np.asarray(Wo), np.asarray(position_query),
                               np.asarray(position_key), np.asarray(bo))
    if "nc" not in _NC_CACHE:
        _NC_CACHE["nc"] = build_nc()
    nc = _NC_CACHE["nc"]
    res = run_bass_kernel_spmd(nc, in_maps, list(range(NC)))
    outs = [res.results[c]["out"] for c in range(NC)]
    return np.concatenate(outs, axis=0)[None].astype(np.float32)  # [1, S, HD]


if __name__ == "__main__":
    inputs = np.load("/tmp/ref_inputs.npy", allow_pickle=True).item()
    out = kernel(**inputs)
    print("kernel out", out.shape, out.dtype)
